# revision 9
# baseline (speedup 1.0000x reference)
"""Trainium2 Bass kernel for a 2-layer GCN (GCNConv+ReLU+BN x2, mean-pool).

Strategy (8 NeuronCores, SPMD):
- Dest-node sharding: each core owns NB=ceil(N/8/128) blocks of 128 nodes.
- Message passing out[c] = sum_e w_e * h[src_e] with w_e = dinv[r]*dinv[c]
  (self-loops included as edges) is FACTORED: stored rows are pre-scaled by
  dinv[src] (on the host for x, on-device for h1), the per-block aggregate
  is post-scaled by dinv[dst] (a per-partition scalar in the psum
  evacuation), so the one-hot matrices E_t[e, d] = (d_e == d) are pure 0/1
  and built with a single VectorEngine is_equal per block (pad slots use
  d=-1 which matches nothing).
- Rows are stored as 133 x fp8e4m3 in 256-byte slots (the dma_gather
  granularity floor), HALVING gather traffic vs fp16; the f32 psum
  accumulation keeps the quantization error ~2e-3 << the 2e-2 gate.
- Aggregation per dest block: psum += E_t^T @ msg_t on the TensorEngine,
  msg_t = 128 source rows fetched with dma_gather.
- Gathers are issued as one continuous tile stream per (core, half): calls
  of 8 tiles (1024 idxs, the SWDGE ring capacity) that cross dest-block
  boundaries; constant full counts (no -1 trimming, no count registers).
  Tiles per (half, block) are the exact max over cores of ceil(edges/128),
  so padding is minimal. Calls round-robin over 4 SWDGE queues.
- The weight multiply commutes with aggregation: per block, agg is
  transposed on the PE and multiplied by W' = W*diag(bn_scale) (fp16, f32
  accum; the BN scale folds into W since relu(z*g) == relu(z)*g for g>0),
  bias added via a ones-row matmul, then ReLU + psum evacuations on the
  otherwise-idle ScalarE, and the BN shift on the VectorE.
- Layer-1 results are AllGathered in CH chunks of NB/CH blocks each so the
  collective overlaps the remaining layer-1 compute; the chunk-major row
  permutation this induces in h1f is baked into the host-side layout of
  xbuf as well, so both layers share a single gather index table.
- Mean-pool: per block a one-hot P matmul reduces 128 nodes into <=128
  graph partials; the host sums overlapping block partials and divides by
  graph counts (the cross-core unshard step).
"""
import os
import numpy as np
import ml_dtypes
from contextlib import ExitStack

import concourse.bacc as bacc
import concourse.bass as bass
import concourse.mybir as mybir
import concourse.tile as tile
from concourse.library_config import mlp
from concourse.bass_utils import run_bass_kernel_spmd

dt = mybir.dt
F8NP = ml_dtypes.float8_e4m3
NCORES = 8
PB = 128          # nodes per dest block
EW = 256          # fp8 elements per padded row (256 bytes)
EPS = 1e-5
G_OUT = 2048      # number of graphs in the output
CH = 7            # AllGather chunks (must divide NB)
# tiles per gather call; the SWDGE ring holds 1024 descriptors per queue,
# so 8 tiles (1024 idxs) is the max call size (larger calls deadlock)
CALL_T = int(os.environ.get("GCN_CALLT", "8"))
SINGLE_PACKET = os.environ.get("GCN_SP", "1") == "1"


# ---------------------------------------------------------------- host prep
def preprocess(x, edge_index, batch):
    N, D = x.shape
    G = G_OUT
    NB = -(-N // (NCORES * PB))          # blocks per core
    assert NB % CH == 0, (NB, CH)
    CB = NB // CH                         # blocks per chunk
    CHS = CB * PB                         # rows per (core, chunk)
    S = NB * PB
    TOT = NCORES * S
    L = TOT // 2
    assert L <= 32768 and TOT - L <= 32768

    r = np.asarray(edge_index[0], dtype=np.int64)
    c = np.asarray(edge_index[1], dtype=np.int64)
    loops = np.arange(N, dtype=np.int64)
    r = np.concatenate([r, loops])
    c = np.concatenate([c, loops])
    deg = np.bincount(c, minlength=N).astype(np.float64)
    dinv_n = np.where(deg > 0, 1.0 / np.sqrt(deg), 0.0).astype(np.float32)
    dinv = np.zeros(TOT, dtype=np.float32)
    dinv[:N] = dinv_n

    # chunk-major row permutation: node n -> storage row pos[n]; matches the
    # layout the chunked AllGather produces for h1f; xbuf uses it too so
    # both layers share one index table.
    nodes = np.arange(TOT, dtype=np.int64)
    k_of = nodes // S
    rloc = nodes % S
    ch_of = rloc // CHS
    pos_all = ch_of * (NCORES * CHS) + k_of * CHS + (rloc % CHS)
    src_pos = pos_all[r]                  # storage row of each edge's source

    half = (src_pos >= L).astype(np.int64)
    src = np.where(half == 0, src_pos, src_pos - L).astype(np.int64)
    blk = c // PB                         # global dest block 0..NCORES*NB-1
    d = (c % PB).astype(np.int64)

    core = blk // NB
    b_loc = blk % NB
    ngroups = NCORES * 2 * NB
    key = (core * 2 + half) * NB + b_loc
    # sort by (group, src): ascending storage rows within each group give
    # the HBM gather far better page locality
    order = np.lexsort((src, key))
    src, d, key = src[order], d[order], key[order]
    counts = np.bincount(key, minlength=ngroups)
    starts = np.concatenate([[0], np.cumsum(counts)])
    cnt_khb = counts.reshape(NCORES, 2, NB)
    # per-(half, block) tile count: exact max over cores (SPMD uniformity)
    TB = np.maximum(1, -(-cnt_khb.max(axis=0) // PB))      # [2, NB]
    off_h = [np.concatenate([[0], np.cumsum(TB[h])]) for h in range(2)]
    LEN = [int(off_h[h][-1]) * PB for h in range(2)]       # idxs per half
    # E-table column offsets per block (half0 tiles then half1)
    TB2 = TB[0] + TB[1]
    eoff = np.concatenate([[0], np.cumsum(TB2)])
    EWID = int(eoff[-1])

    idx_arr = np.zeros((NCORES, 128, (LEN[0] + LEN[1]) // 16), dtype=np.int16)
    d_arr = np.full((NCORES, 128, EWID), -1.0, dtype=np.float16)
    dinvc_arr = np.zeros((NCORES, 128, NB), dtype=np.float32)
    for k in range(NCORES):
        dinvc_arr[k] = dinv[k * S:(k + 1) * S].reshape(NB, PB).T
        col0 = 0
        for h in range(2):
            stream = np.zeros(LEN[h], dtype=np.int16)
            p0 = 0
            for b in range(NB):
                g = (k * 2 + h) * NB + b
                n = counts[g]
                gsz = int(TB[h][b]) * PB
                sl = src[starts[g]:starts[g] + n]
                seg = np.zeros(gsz, dtype=np.int16)
                seg[:n] = sl
                if n < gsz:
                    seg[n:] = sl[-1] if n > 0 else 0
                stream[p0:p0 + gsz] = seg
                p0 += gsz
                # d columns for this (b, h)
                dseg = np.full(gsz, -1.0, dtype=np.float16)
                dseg[:n] = d[starts[g]:starts[g] + n]
                ecol = int(eoff[b]) + (int(TB[0][b]) if h == 1 else 0)
                d_arr[k, :, ecol:ecol + int(TB[h][b])] = \
                    dseg.reshape(int(TB[h][b]), PB).T
            wrapped = stream.reshape(LEN[h] // 16, 16).T     # [16, LEN/16]
            idx_arr[k, :, col0:col0 + LEN[h] // 16] = np.tile(wrapped, (8, 1))
            col0 += LEN[h] // 16

    # pooling: per (core, block) graph base + local graph ids
    batch_pad = np.full(TOT, -1, dtype=np.int64)
    batch_pad[:N] = np.asarray(batch, dtype=np.int64)
    blocks = batch_pad.reshape(NCORES * NB, PB)
    valid = blocks >= 0
    base = np.where(valid.any(axis=1),
                    np.where(valid, blocks, np.iinfo(np.int64).max).min(axis=1),
                    0)
    bloc = np.where(valid, blocks - base[:, None], -1).astype(np.float32)
    bloc_arr = bloc.reshape(NCORES, NB, PB).transpose(0, 2, 1).copy()
    gids = np.arange(PB, dtype=np.float32)
    p_arr = (bloc_arr[:, :, :, None] == gids).astype(np.float16)
    p_arr = p_arr.reshape(NCORES, 128, NB * PB)

    cnts = np.bincount(np.asarray(batch, dtype=np.int64),
                       minlength=G).astype(np.float32)
    return dict(N=N, D=D, G=G, NB=NB, S=S, TOT=TOT, L=L, CHS=CHS,
                TB=TB, LEN=LEN, eoff=eoff, EWID=EWID, pos_all=pos_all,
                dinv=dinv, idx_arr=idx_arr, d_arr=d_arr,
                dinvc_arr=dinvc_arr, p_arr=p_arr, base=base, cnts=cnts)


def fold_bn(g, beta, rm, rv):
    gp = (g / np.sqrt(rv + EPS)).astype(np.float32)
    bp = (beta - rm * gp).astype(np.float32)
    return gp, bp


# ---------------------------------------------------------------- bass build
def build_nc(NB, D, TOT, S, L, TB0, TB1, reps=1):
    f8, f16, f32, i16 = dt.float8e4, dt.float16, dt.float32, dt.int16
    NQ = int(os.environ.get("GCN_NQ", "4"))      # SWDGE queues (Q7 cpu pairs)
    SCR = int(os.environ.get("GCN_SCRATCH", "16384"))
    CB = NB // CH
    CHS = CB * PB
    TB = np.stack([np.asarray(TB0), np.asarray(TB1)])
    off_h = [np.concatenate([[0], np.cumsum(TB[h])]) for h in range(2)]
    LEN = [int(off_h[h][-1]) * PB for h in range(2)]
    TB2 = TB[0] + TB[1]
    eoff = np.concatenate([[0], np.cumsum(TB2)])
    EWID = int(eoff[-1])
    nc = bacc.Bacc("TRN2", target_bir_lowering=False, debug=False,
                   num_devices=NCORES, num_swdge_queues=NQ,
                   dynamic_dma_scratch_size=SCR)

    xbuf = nc.dram_tensor("xbuf", [TOT, EW], f8, kind="ExternalInput")
    idxt = nc.dram_tensor("idxt", [128, (LEN[0] + LEN[1]) // 16], i16,
                          kind="ExternalInput")
    dcol = nc.dram_tensor("dcol", [128, EWID], f16, kind="ExternalInput")
    dvc = nc.dram_tensor("dvc", [128, NB], f32, kind="ExternalInput")
    dvc16 = nc.dram_tensor("dvc16", [128, NB], f16, kind="ExternalInput")
    pcol = nc.dram_tensor("pcol", [128, NB * 128], f16, kind="ExternalInput")
    iot = nc.dram_tensor("iot", [128, 128], f16, kind="ExternalInput")
    idn = nc.dram_tensor("idn", [128, 128], f32, kind="ExternalInput")
    onesr = nc.dram_tensor("onesr", [1, 128], f16, kind="ExternalInput")
    whi = nc.dram_tensor("whi", [2, 128, D], f16, kind="ExternalInput")
    wlo = nc.dram_tensor("wlo", [2, D - 128, D], f16, kind="ExternalInput")
    brow = nc.dram_tensor("brow", [2, 1, D], f16, kind="ExternalInput")
    bet = nc.dram_tensor("bet", [2, 128, D], f32, kind="ExternalInput")
    outp = nc.dram_tensor("outp", [NB * PB, D], f32, kind="ExternalOutput")
    h1sl = nc.dram_tensor("h1sl", [S, EW], f8, kind="Internal")
    h1f = nc.dram_tensor("h1f", [TOT, EW], f8, kind="Internal",
                         addr_space="Shared")

    DLO = D - 128
    with tile.TileContext(nc) as tc, ExitStack() as ctx:
        cp = ctx.enter_context(tc.tile_pool(name="consts", bufs=1))
        gp_ = ctx.enter_context(tc.tile_pool(name="gath", bufs=12))
        ep = ctx.enter_context(tc.tile_pool(name="onehot", bufs=2))
        sp = ctx.enter_context(tc.tile_pool(name="stage", bufs=3))
        pagg = ctx.enter_context(tc.tile_pool(name="pagg", bufs=2, space="PSUM"))
        ptr = ctx.enter_context(tc.tile_pool(name="ptr", bufs=2, space="PSUM"))
        ph = ctx.enter_context(tc.tile_pool(name="ph", bufs=2, space="PSUM"))
        ppool = ctx.enter_context(tc.tile_pool(name="ppool", bufs=2, space="PSUM"))

        def load_const(name, dram, shape, dtype):
            t = cp.tile(shape, dtype, name=name)
            nc.sync.dma_start(t[:], dram)
            return t

        idxS = load_const("idxS", idxt[:, :], [128, (LEN[0] + LEN[1]) // 16], i16)
        dS = load_const("dS", dcol[:, :], [128, EWID], f16)
        dvcS = load_const("dvcS", dvc[:, :], [128, NB], f32)
        dvc16S = load_const("dvc16S", dvc16[:, :], [128, NB], f16)
        pS = load_const("pS", pcol[:, :], [128, NB * 128], f16)
        iotS = load_const("iotS", iot[:, :], [128, 128], f16)
        idnS = load_const("idnS", idn[:, :], [128, 128], f32)
        onesS = load_const("onesS", onesr[:, :], [1, 128], f16)
        whiS = [load_const(f"whiS{l}", whi[l, :, :], [128, D], f16) for l in range(2)]
        wloS = [load_const(f"wloS{l}", wlo[l, :, :], [DLO, D], f16) for l in range(2)]
        browS = [load_const(f"browS{l}", brow[l, :, :], [1, D], f16) for l in range(2)]
        betS = [load_const(f"betS{l}", bet[l, :, :], [128, D], f32) for l in range(2)]

        nc.gpsimd.load_library(mlp)

        env = dict(NB=NB, D=D, TOT=TOT, S=S, L=L, NQ=NQ,
                   TB=TB, off_h=off_h, LEN=LEN, eoff=eoff, CB=CB, CHS=CHS,
                   xbuf=xbuf, h1f=h1f, h1sl=h1sl, outp=outp,
                   gp_=gp_, ep=ep, sp=sp,
                   pagg=pagg, ptr=ptr, ph=ph, ppool=ppool,
                   idxS=idxS, dS=dS, dvcS=dvcS, dvc16S=dvc16S, pS=pS,
                   iotS=iotS, idnS=idnS,
                   onesS=onesS, whiS=whiS, wloS=wloS, browS=browS,
                   betS=betS)
        for rep in range(reps):
            env["rep"] = rep
            _do_body(nc, tc, env)

    nc.compile()
    return nc


def _do_body(nc, tc, env):
    """One full forward pass; env carries the tiles/pools from build_nc."""
    (NB, D, TOT, S, L, NQ, CB, CHS, rep) = (env[k] for k in
        ("NB", "D", "TOT", "S", "L", "NQ", "CB", "CHS", "rep"))
    (TB, off_h, LEN, eoff) = (env[k] for k in ("TB", "off_h", "LEN", "eoff"))
    (xbuf, h1f, h1sl, outp) = (env[k] for k in ("xbuf", "h1f", "h1sl", "outp"))
    (gp_, ep, sp, pagg, ptr, ph, ppool) = (env[k] for k in
        ("gp_", "ep", "sp", "pagg", "ptr", "ph", "ppool"))
    (idxS, dS, dvcS, dvc16S, pS, iotS, idnS, onesS) = (env[k] for k in
        ("idxS", "dS", "dvcS", "dvc16S", "pS", "iotS", "idnS", "onesS"))
    (whiS, wloS, browS, betS) = (env[k] for k in
        ("whiS", "wloS", "browS", "betS"))
    f8, f16, f32 = dt.float8e4, dt.float16, dt.float32
    DLO = D - 128
    R = rep
    TT = [int(off_h[0][-1]), int(off_h[1][-1])]   # tiles per half stream
    idx_base = [0, LEN[0] // 16]
    qn = [0]
    for layer in range(2):
        src = xbuf if layer == 0 else h1f
        issued = [0, 0]           # tiles issued so far per half
        call_of = [{}, {}]        # tile index -> (call tile-base, gt tile)
        def issue_calls(h, upto):
            while issued[h] < upto:
                t0 = issued[h]
                tcn = min(CALL_T, TT[h] - t0)
                gt = gp_.tile([128, tcn, EW], f8,
                              name=f"gt_{R}_{layer}_{h}_{t0}", tag="gt")
                in_ap = src[0:TOT, :] if h == 0 else src[L:TOT, :]
                nc.gpsimd.dma_gather(
                    gt[:], in_ap,
                    idxS[:, idx_base[h] + t0 * 8:
                         idx_base[h] + (t0 + tcn) * 8],
                    tcn * PB, tcn * PB, EW, single_packet=SINGLE_PACKET,
                    queue_num=qn[0] % NQ)
                qn[0] += 1
                for t in range(t0, t0 + tcn):
                    call_of[h][t] = (t0, gt)
                issued[h] += tcn
        for b in range(NB):
            tb0, tb1 = int(TB[0][b]), int(TB[1][b])
            tb2 = tb0 + tb1
            ecol = int(eoff[b])
            agg = pagg.tile([128, D], f32, name=f"agg_{R}_{layer}_{b}",
                            tag="agg")
            E = ep.tile([128, tb2, 128], f16,
                        name=f"E_{R}_{layer}_{b}", tag="E")
            iotB = iotS[:, :].unsqueeze(1).broadcast_to((128, tb2, 128))
            dB = dS[:, ecol:ecol + tb2].unsqueeze(2) \
                .broadcast_to((128, tb2, 128))
            nc.vector.tensor_tensor(E[:], iotB, dB,
                                    op=mybir.AluOpType.is_equal)
            nmm = tb2
            mi = 0
            for h in range(2):
                sbase = int(off_h[h][b])
                tbh = tb0 if h == 0 else tb1
                issue_calls(h, sbase + tbh)
                for t in range(tbh):
                    t0, gt = call_of[h][sbase + t]
                    nc.tensor.matmul(
                        agg[:], E[:, (0 if h == 0 else tb0) + t, :],
                        gt[:, sbase + t - t0, 0:D],
                        start=(mi == 0), stop=(mi == nmm - 1))
                    mi += 1
                for t in range(sbase, sbase + tbh):
                    call_of[h].pop(t, None)
            # epilogue: dinv[dst] post-scale + transpose, @W, +b, relu, BN
            aggS = sp.tile([128, D], f32, name=f"aggS_{R}_{layer}_{b}",
                           tag="aggS")
            nc.scalar.activation(aggS[:], agg[:],
                                 mybir.ActivationFunctionType.Copy,
                                 scale=dvcS[:, b:b + 1])
            psT = ptr.tile([128, 256], f32, name=f"psT_{R}_{layer}_{b}",
                           tag="psT")
            nc.tensor.transpose(psT[:, 0:128], aggS[:, 0:128], idnS[:])
            nc.tensor.transpose(psT[0:DLO, 128:256], aggS[:, 128:D], idnS[:])
            t1 = sp.tile([128, 128], f16, name=f"t1_{R}_{layer}_{b}", tag="t1")
            nc.scalar.activation(t1[:], psT[:, 0:128],
                                 mybir.ActivationFunctionType.Copy)
            t2 = sp.tile([DLO, 128], f16, name=f"t2_{R}_{layer}_{b}", tag="t2")
            nc.scalar.activation(t2[:], psT[0:DLO, 128:256],
                                 mybir.ActivationFunctionType.Copy)
            zps = ph.tile([128, D], f32, name=f"zps_{R}_{layer}_{b}", tag="zps")
            nc.tensor.matmul(zps[:], t1[:], whiS[layer][:],
                             start=True, stop=False)
            nc.tensor.matmul(zps[:], t2[:], wloS[layer][:],
                             start=False, stop=False)
            nc.tensor.matmul(zps[:], onesS[:], browS[layer][:],
                             start=False, stop=True)
            rl = sp.tile([128, D], f32, name=f"rl_{R}_{layer}_{b}", tag="rl")
            nc.scalar.activation(rl[:], zps[:],
                                 mybir.ActivationFunctionType.Relu)
            hS = sp.tile([128, D], f16, name=f"hS_{R}_{layer}_{b}", tag="hS")
            nc.vector.tensor_add(hS[:], rl[:], betS[layer][:])
            if layer == 0:
                # store row = dinv[dst] * h1, packed to fp8 in a 256B slot
                # (ScalarE does the scale+cast; DVE f8 casts are very slow)
                r1 = sp.tile([128, EW], f8, name=f"r1_{R}_{b}", tag="r1")
                nc.scalar.activation(r1[:, 0:D], hS[:],
                                     mybir.ActivationFunctionType.Copy,
                                     scale=dvcS[:, b:b + 1])
                nc.sync.dma_start(h1sl[b * PB:(b + 1) * PB, 0:D], r1[:, 0:D])
                if (b + 1) % CB == 0:
                    ch = b // CB
                    nc.gpsimd.collective_compute(
                        "AllGather", mybir.AluOpType.bypass,
                        replica_groups=[list(range(NCORES))],
                        ins=[h1sl[ch * CHS:(ch + 1) * CHS, :].opt()],
                        outs=[h1f[ch * NCORES * CHS:
                                  (ch + 1) * NCORES * CHS, :].opt()])
            else:
                pps = ppool.tile([128, D], f32, name=f"pps_{R}_{b}", tag="pps")
                nc.tensor.matmul(pps[:], pS[:, b * 128:(b + 1) * 128], hS[:],
                                 start=True, stop=True)
                po = sp.tile([128, D], f32, name=f"po_{R}_{b}", tag="po")
                nc.scalar.activation(po[:], pps[:],
                                     mybir.ActivationFunctionType.Copy)
                nc.sync.dma_start(outp[b * PB:(b + 1) * PB, :], po[:])


# ---------------------------------------------------------------- entry
_NC_CACHE = {}


def kernel(x, edge_index, batch, W1, b1, W2, b2,
           g1, beta1, rm1, rv1, g2, beta2, rm2, rv2):
    nc, in_maps, pp = prepare(x, edge_index, batch, W1, b1, W2, b2,
                              g1, beta1, rm1, rv1, g2, beta2, rm2, rv2)
    res = run_bass_kernel_spmd(nc, in_maps, core_ids=list(range(NCORES)))
    return combine(pp, [res.results[k]["outp"] for k in range(NCORES)])


def prepare(x, edge_index, batch, W1, b1, W2, b2,
            g1, beta1, rm1, rv1, g2, beta2, rm2, rv2):
    """Build (nc, in_maps, pp) without running — used by the benchmark."""
    x = np.asarray(x, dtype=np.float32)
    pp = preprocess(x, np.asarray(edge_index), np.asarray(batch))
    D = pp["D"]
    key = (pp["NB"], D, pp["TOT"], pp["S"], pp["L"],
           tuple(int(v) for v in pp["TB"][0]),
           tuple(int(v) for v in pp["TB"][1]))
    if key not in _NC_CACHE:
        _NC_CACHE[key] = build_nc(*key)
    nc = _NC_CACHE[key]

    N = pp["N"]
    xbuf = np.zeros((pp["TOT"], EW), dtype=F8NP)
    xs = (x * pp["dinv"][:N, None]).astype(np.float16).astype(F8NP)
    xbuf[pp["pos_all"][:N], :D] = xs
    iot = np.broadcast_to(np.arange(128, dtype=np.float16), (128, 128)).copy()
    idn = np.eye(128, dtype=np.float32)
    onesr = np.ones((1, 128), dtype=np.float16)
    g1p, b1p = fold_bn(g1, beta1, rm1, rv1)
    g2p, b2p = fold_bn(g2, beta2, rm2, rv2)
    assert (g1p > 0).all() and (g2p > 0).all(), \
        "gamma fold into W needs positive BN scale (relu(z*g) == relu(z)*g)"
    whi = np.stack([W1[:128] * g1p, W2[:128] * g2p]).astype(np.float16)
    wlo = np.stack([W1[128:] * g1p, W2[128:] * g2p]).astype(np.float16)
    brow = np.stack([(b1 * g1p)[None, :],
                     (b2 * g2p)[None, :]]).astype(np.float16)
    bet = np.stack([np.broadcast_to(b1p, (128, D)),
                    np.broadcast_to(b2p, (128, D))]).astype(np.float32)
    in_maps = []
    for k in range(NCORES):
        in_maps.append({
            "xbuf": xbuf, "idxt": pp["idx_arr"][k], "dcol": pp["d_arr"][k],
            "dvc": pp["dinvc_arr"][k],
            "dvc16": pp["dinvc_arr"][k].astype(np.float16),
            "pcol": pp["p_arr"][k],
            "iot": iot, "idn": idn, "onesr": onesr,
            "whi": whi, "wlo": wlo, "brow": brow, "bet": bet,
        })
    return nc, in_maps, pp


def combine(pp, outs):
    sums = np.zeros((pp["G"] + PB, pp["D"]), dtype=np.float32)
    for k in range(NCORES):
        o = outs[k]
        for b in range(pp["NB"]):
            bb = pp["base"][k * pp["NB"] + b]
            sums[bb:bb + PB] += o[b * PB:(b + 1) * PB]
    return (sums[:pp["G"]]
            / np.maximum(pp["cnts"], 1.0)[:, None]).astype(np.float32)


# revision 12
# speedup vs baseline: 1.2035x; 1.2035x over previous
"""Trainium2 Bass kernel for a 2-layer GCN (GCNConv+ReLU+BN x2, mean-pool).

Strategy (8 NeuronCores, SPMD):
- Dest-node sharding: each core owns NB=ceil(N/8/128) blocks of 128 nodes.
- Message passing out[c] = sum_e w_e * h[src_e] with w_e = dinv[r]*dinv[c]
  is FACTORED: stored rows are pre-scaled by dinv[src] (on the host for x,
  on-device for h1), the per-block aggregate is post-scaled by dinv[dst]
  (a per-partition scalar in the psum evacuation), so the one-hot matrices
  E_t[e, d] = (d_e == d) are pure 0/1 and built with a single VectorEngine
  is_equal per block (pad slots use d=-1 which matches nothing).
- Rows are stored as 133 x fp8e4m3 in 256-byte slots (the dma_gather
  granularity floor), HALVING gather traffic vs fp16; the f32 psum
  accumulation keeps the quantization error ~2e-3 << the 2e-2 gate.
- Self-loops are NOT edges: each block's own 128 rows are fetched with one
  contiguous HWDGE DMA (layer 1 from a per-core xown copy, layer 2 from
  the core's own h1sl) and added into the psum via an fp8 identity matmul
  that also opens the accumulation group; the dinv[dst] post-scale turns
  the pre-scaled own row into the exact dinv^2 self-loop term.
- Aggregation per dest block: psum += E_t^T @ msg_t on the TensorEngine,
  msg_t = 128 source rows fetched with dma_gather.
- Gathers are issued as one continuous tile stream per (core, half): calls
  of 8 tiles (1024 idxs, the SWDGE ring capacity; 2048 deadlocks) that
  cross dest-block boundaries; constant full counts (no -1 trimming, no
  count registers). Tiles per (half, block) are the exact max over cores
  of ceil(edges/128). Calls round-robin over 4 SWDGE queues. Each call's
  indices are rebased to the call's (cross-core) min row and its in_ap is
  sliced to [lo, hi) so layer-2 calls only depend on the h1f chunks they
  actually read.
- The weight multiply commutes with aggregation: per block, agg is
  transposed on the PE and multiplied by W' = W*diag(bn_scale) (fp16, f32
  accum; the BN scale folds into W since relu(z*g) == relu(z)*g for g>0),
  bias added via a ones-row matmul, then ReLU + psum evacuations on the
  otherwise-idle ScalarE, and the BN shift on the VectorE.
- Layer-1 results are AllGathered in uneven chunks (8,8,...,1 blocks) so
  the collective overlaps the remaining layer-1 compute and only the tiny
  last chunk's latency is exposed; the chunk-major row permutation this
  induces in h1f is baked into the host-side layout of xbuf too, so both
  layers share a single gather index table.
- Mean-pool: per block a one-hot P matmul reduces 128 nodes into <=128
  graph partials; the host sums overlapping block partials and divides by
  graph counts (the cross-core unshard step).
"""
import os
import numpy as np
import ml_dtypes
from contextlib import ExitStack

import concourse.bacc as bacc
import concourse.bass as bass
import concourse.mybir as mybir
import concourse.tile as tile
from concourse.library_config import mlp
from concourse.bass_utils import run_bass_kernel_spmd

dt = mybir.dt
F8NP = ml_dtypes.float8_e4m3
NCORES = 8
PB = 128          # nodes per dest block
EW = 256          # fp8 elements per padded row (256 bytes)
EPS = 1e-5
G_OUT = 2048      # number of graphs in the output
CBL = [8, 8, 8, 8, 8, 8, 1]   # blocks per AllGather chunk (sums to NB)
# tiles per gather call; the SWDGE ring holds 1024 descriptors per queue,
# so 8 tiles (1024 idxs) is the max call size (larger calls deadlock)
CALL_T = int(os.environ.get("GCN_CALLT", "8"))
SINGLE_PACKET = os.environ.get("GCN_SP", "1") == "1"


# ---------------------------------------------------------------- host prep
def preprocess(x, edge_index, batch):
    N, D = x.shape
    G = G_OUT
    NB = -(-N // (NCORES * PB))          # blocks per core
    assert sum(CBL) == NB, (NB, CBL)
    cumr = np.concatenate([[0], np.cumsum(np.asarray(CBL) * PB)])  # per-core
    S = NB * PB
    TOT = NCORES * S
    L = TOT // 2
    assert L <= 32768 and TOT - L <= 32768

    r = np.asarray(edge_index[0], dtype=np.int64)
    c = np.asarray(edge_index[1], dtype=np.int64)
    deg = (np.bincount(c, minlength=N) + 1).astype(np.float64)  # + self-loop
    dinv_n = (1.0 / np.sqrt(deg)).astype(np.float32)
    dinv = np.zeros(TOT, dtype=np.float32)
    dinv[:N] = dinv_n

    # chunk-major row permutation: node n -> storage row pos[n]; matches the
    # layout the chunked AllGather produces for h1f; xbuf uses it too so
    # both layers share one index table.
    nodes = np.arange(TOT, dtype=np.int64)
    k_of = nodes // S
    rloc = nodes % S
    ch_of = np.searchsorted(cumr, rloc, side="right") - 1
    chrows = (cumr[ch_of + 1] - cumr[ch_of])
    pos_all = (NCORES * cumr[ch_of] + k_of * chrows
               + (rloc - cumr[ch_of]))
    src_pos = pos_all[r]                  # storage row of each edge's source

    half = (src_pos >= L).astype(np.int64)
    src = np.where(half == 0, src_pos, src_pos - L).astype(np.int64)
    blk = c // PB                         # global dest block 0..NCORES*NB-1
    d = (c % PB).astype(np.int64)

    core = blk // NB
    b_loc = blk % NB
    ngroups = NCORES * 2 * NB
    key = (core * 2 + half) * NB + b_loc
    # sort by (group, src): ascending storage rows within each group give
    # the HBM gather far better page locality (and tight per-call ranges)
    order = np.lexsort((src, key))
    src, d, key = src[order], d[order], key[order]
    counts = np.bincount(key, minlength=ngroups)
    starts = np.concatenate([[0], np.cumsum(counts)])
    cnt_khb = counts.reshape(NCORES, 2, NB)
    # per-(half, block) tile count: exact max over cores (SPMD uniformity)
    TB = np.maximum(1, -(-cnt_khb.max(axis=0) // PB))      # [2, NB]
    off_h = [np.concatenate([[0], np.cumsum(TB[h])]) for h in range(2)]
    LEN = [int(off_h[h][-1]) * PB for h in range(2)]       # idxs per half
    TB2 = TB[0] + TB[1]
    eoff = np.concatenate([[0], np.cumsum(TB2)])
    EWID = int(eoff[-1])

    # raw (unrebased) idx streams + d tables
    streams = np.zeros((NCORES, 2, max(LEN)), dtype=np.int32)
    d_arr = np.full((NCORES, 128, EWID), -1.0, dtype=np.float16)
    dinvc_arr = np.zeros((NCORES, 128, NB), dtype=np.float32)
    for k in range(NCORES):
        dinvc_arr[k] = dinv[k * S:(k + 1) * S].reshape(NB, PB).T
        for h in range(2):
            p0 = 0
            for b in range(NB):
                g = (k * 2 + h) * NB + b
                n = counts[g]
                gsz = int(TB[h][b]) * PB
                sl = src[starts[g]:starts[g] + n]
                seg = np.zeros(gsz, dtype=np.int32)
                seg[:n] = sl
                if n < gsz:
                    seg[n:] = sl[-1] if n > 0 else 0
                streams[k, h, p0:p0 + gsz] = seg
                p0 += gsz
                dseg = np.full(gsz, -1.0, dtype=np.float16)
                dseg[:n] = d[starts[g]:starts[g] + n]
                ecol = int(eoff[b]) + (int(TB[0][b]) if h == 1 else 0)
                d_arr[k, :, ecol:ecol + int(TB[h][b])] = \
                    dseg.reshape(int(TB[h][b]), PB).T

    # per-call cross-core [lo, hi) row ranges; rebase idxs to lo
    call_lo, call_hi = [[], []], [[], []]
    idx_arr = np.zeros((NCORES, 128, (LEN[0] + LEN[1]) // 16), dtype=np.int16)
    for h in range(2):
        TTh = LEN[h] // PB
        t0 = 0
        while t0 < TTh:
            tcn = min(CALL_T, TTh - t0)
            sl = streams[:, h, t0 * PB:(t0 + tcn) * PB]
            lo = int(sl.min())
            hi = int(sl.max()) + 1
            assert hi - lo <= 32768
            call_lo[h].append(lo)
            call_hi[h].append(hi)
            t0 += tcn
    for k in range(NCORES):
        col0 = 0
        for h in range(2):
            TTh = LEN[h] // PB
            reb = np.zeros(LEN[h], dtype=np.int16)
            t0 = 0
            ci = 0
            while t0 < TTh:
                tcn = min(CALL_T, TTh - t0)
                seg = streams[k, h, t0 * PB:(t0 + tcn) * PB]
                reb[t0 * PB:(t0 + tcn) * PB] = seg - call_lo[h][ci]
                ci += 1
                t0 += tcn
            wrapped = reb.reshape(LEN[h] // 16, 16).T
            idx_arr[k, :, col0:col0 + LEN[h] // 16] = np.tile(wrapped, (8, 1))
            col0 += LEN[h] // 16

    # pooling: per (core, block) graph base + local graph ids
    batch_pad = np.full(TOT, -1, dtype=np.int64)
    batch_pad[:N] = np.asarray(batch, dtype=np.int64)
    blocks = batch_pad.reshape(NCORES * NB, PB)
    valid = blocks >= 0
    base = np.where(valid.any(axis=1),
                    np.where(valid, blocks, np.iinfo(np.int64).max).min(axis=1),
                    0)
    bloc = np.where(valid, blocks - base[:, None], -1).astype(np.float32)
    bloc_arr = bloc.reshape(NCORES, NB, PB).transpose(0, 2, 1).copy()
    gids = np.arange(PB, dtype=np.float32)
    p_arr = (bloc_arr[:, :, :, None] == gids).astype(np.float16)
    p_arr = p_arr.reshape(NCORES, 128, NB * PB)

    cnts = np.bincount(np.asarray(batch, dtype=np.int64),
                       minlength=G).astype(np.float32)
    return dict(N=N, D=D, G=G, NB=NB, S=S, TOT=TOT, L=L, cumr=cumr,
                TB=TB, LEN=LEN, eoff=eoff, EWID=EWID, pos_all=pos_all,
                dinv=dinv, idx_arr=idx_arr, d_arr=d_arr,
                call_lo=call_lo, call_hi=call_hi,
                dinvc_arr=dinvc_arr, p_arr=p_arr, base=base, cnts=cnts)


def fold_bn(g, beta, rm, rv):
    gp = (g / np.sqrt(rv + EPS)).astype(np.float32)
    bp = (beta - rm * gp).astype(np.float32)
    return gp, bp


# ---------------------------------------------------------------- bass build
def build_nc(NB, D, TOT, S, L, TB0, TB1, CLO0, CHI0, CLO1, CHI1, reps=1):
    f8, f16, f32, i16 = dt.float8e4, dt.float16, dt.float32, dt.int16
    NQ = int(os.environ.get("GCN_NQ", "4"))      # SWDGE queues (Q7 cpu pairs)
    SCR = int(os.environ.get("GCN_SCRATCH", "16384"))
    cumr = np.concatenate([[0], np.cumsum(np.asarray(CBL) * PB)])
    TB = np.stack([np.asarray(TB0), np.asarray(TB1)])
    off_h = [np.concatenate([[0], np.cumsum(TB[h])]) for h in range(2)]
    LEN = [int(off_h[h][-1]) * PB for h in range(2)]
    TB2 = TB[0] + TB[1]
    eoff = np.concatenate([[0], np.cumsum(TB2)])
    EWID = int(eoff[-1])
    call_lo, call_hi = [CLO0, CLO1], [CHI0, CHI1]
    nc = bacc.Bacc("TRN2", target_bir_lowering=False, debug=False,
                   num_devices=NCORES, num_swdge_queues=NQ,
                   dynamic_dma_scratch_size=SCR)

    xbuf = nc.dram_tensor("xbuf", [TOT, EW], f8, kind="ExternalInput")
    xown = nc.dram_tensor("xown", [S, EW], f8, kind="ExternalInput")
    idxt = nc.dram_tensor("idxt", [128, (LEN[0] + LEN[1]) // 16], i16,
                          kind="ExternalInput")
    dcol = nc.dram_tensor("dcol", [128, EWID], f16, kind="ExternalInput")
    dvc = nc.dram_tensor("dvc", [128, NB], f32, kind="ExternalInput")
    pcol = nc.dram_tensor("pcol", [128, NB * 128], f16, kind="ExternalInput")
    iot = nc.dram_tensor("iot", [128, 128], f16, kind="ExternalInput")
    idn = nc.dram_tensor("idn", [128, 128], f32, kind="ExternalInput")
    idn8 = nc.dram_tensor("idn8", [128, 128], f8, kind="ExternalInput")
    onesr = nc.dram_tensor("onesr", [1, 128], f16, kind="ExternalInput")
    whi = nc.dram_tensor("whi", [2, 128, D], f16, kind="ExternalInput")
    wlo = nc.dram_tensor("wlo", [2, D - 128, D], f16, kind="ExternalInput")
    brow = nc.dram_tensor("brow", [2, 1, D], f16, kind="ExternalInput")
    bet = nc.dram_tensor("bet", [2, 128, D], f32, kind="ExternalInput")
    outp = nc.dram_tensor("outp", [NB * PB, D], f32, kind="ExternalOutput")
    h1sl = nc.dram_tensor("h1sl", [S, EW], f8, kind="Internal")
    h1f = nc.dram_tensor("h1f", [TOT, EW], f8, kind="Internal",
                         addr_space="Shared")

    DLO = D - 128
    with tile.TileContext(nc) as tc, ExitStack() as ctx:
        cp = ctx.enter_context(tc.tile_pool(name="consts", bufs=1))
        gp_ = ctx.enter_context(tc.tile_pool(name="gath", bufs=12))
        dg = ctx.enter_context(tc.tile_pool(name="diag", bufs=3))
        ep = ctx.enter_context(tc.tile_pool(name="onehot", bufs=2))
        sp = ctx.enter_context(tc.tile_pool(name="stage", bufs=3))
        pagg = ctx.enter_context(tc.tile_pool(name="pagg", bufs=2, space="PSUM"))
        ptr = ctx.enter_context(tc.tile_pool(name="ptr", bufs=2, space="PSUM"))
        ph = ctx.enter_context(tc.tile_pool(name="ph", bufs=2, space="PSUM"))
        ppool = ctx.enter_context(tc.tile_pool(name="ppool", bufs=2, space="PSUM"))

        def load_const(name, dram, shape, dtype):
            t = cp.tile(shape, dtype, name=name)
            nc.sync.dma_start(t[:], dram)
            return t

        idxS = load_const("idxS", idxt[:, :], [128, (LEN[0] + LEN[1]) // 16], i16)
        dS = load_const("dS", dcol[:, :], [128, EWID], f16)
        dvcS = load_const("dvcS", dvc[:, :], [128, NB], f32)
        pS = load_const("pS", pcol[:, :], [128, NB * 128], f16)
        iotS = load_const("iotS", iot[:, :], [128, 128], f16)
        idnS = load_const("idnS", idn[:, :], [128, 128], f32)
        idn8S = load_const("idn8S", idn8[:, :], [128, 128], f8)
        onesS = load_const("onesS", onesr[:, :], [1, 128], f16)
        whiS = [load_const(f"whiS{l}", whi[l, :, :], [128, D], f16) for l in range(2)]
        wloS = [load_const(f"wloS{l}", wlo[l, :, :], [DLO, D], f16) for l in range(2)]
        browS = [load_const(f"browS{l}", brow[l, :, :], [1, D], f16) for l in range(2)]
        betS = [load_const(f"betS{l}", bet[l, :, :], [128, D], f32) for l in range(2)]

        nc.gpsimd.load_library(mlp)

        env = dict(NB=NB, D=D, TOT=TOT, S=S, L=L, NQ=NQ, cumr=cumr,
                   TB=TB, off_h=off_h, LEN=LEN, eoff=eoff,
                   call_lo=call_lo, call_hi=call_hi,
                   xbuf=xbuf, xown=xown, h1f=h1f, h1sl=h1sl, outp=outp,
                   gp_=gp_, dg=dg, ep=ep, sp=sp,
                   pagg=pagg, ptr=ptr, ph=ph, ppool=ppool,
                   idxS=idxS, dS=dS, dvcS=dvcS, pS=pS,
                   iotS=iotS, idnS=idnS, idn8S=idn8S,
                   onesS=onesS, whiS=whiS, wloS=wloS, browS=browS,
                   betS=betS)
        for rep in range(reps):
            env["rep"] = rep
            _do_body(nc, tc, env)

    nc.compile()
    return nc


def _do_body(nc, tc, env):
    """One full forward pass; env carries the tiles/pools from build_nc."""
    (NB, D, TOT, S, L, NQ, cumr, rep) = (env[k] for k in
        ("NB", "D", "TOT", "S", "L", "NQ", "cumr", "rep"))
    (TB, off_h, LEN, eoff) = (env[k] for k in ("TB", "off_h", "LEN", "eoff"))
    (call_lo, call_hi) = (env[k] for k in ("call_lo", "call_hi"))
    (xbuf, xown, h1f, h1sl, outp) = (env[k] for k in
        ("xbuf", "xown", "h1f", "h1sl", "outp"))
    (gp_, dg, ep, sp, pagg, ptr, ph, ppool) = (env[k] for k in
        ("gp_", "dg", "ep", "sp", "pagg", "ptr", "ph", "ppool"))
    (idxS, dS, dvcS, pS, iotS, idnS, idn8S, onesS) = (env[k] for k in
        ("idxS", "dS", "dvcS", "pS", "iotS", "idnS", "idn8S", "onesS"))
    (whiS, wloS, browS, betS) = (env[k] for k in
        ("whiS", "wloS", "browS", "betS"))
    f8, f16, f32 = dt.float8e4, dt.float16, dt.float32
    DLO = D - 128
    R = rep
    TT = [int(off_h[0][-1]), int(off_h[1][-1])]   # tiles per half stream
    idx_base = [0, LEN[0] // 16]
    qn = [0]
    for layer in range(2):
        src = xbuf if layer == 0 else h1f
        own = xown if layer == 0 else h1sl
        issued = [0, 0]           # tiles issued so far per half
        ncall = [0, 0]
        call_of = [{}, {}]        # tile index -> (call tile-base, gt tile)
        def issue_calls(h, upto):
            while issued[h] < upto:
                t0 = issued[h]
                tcn = min(CALL_T, TT[h] - t0)
                ci = ncall[h]
                lo, hi = call_lo[h][ci], call_hi[h][ci]
                base = 0 if h == 0 else L
                gt = gp_.tile([128, tcn, EW], f8,
                              name=f"gt_{R}_{layer}_{h}_{t0}", tag="gt")
                nc.gpsimd.dma_gather(
                    gt[:], src[base + lo:base + hi, :],
                    idxS[:, idx_base[h] + t0 * 8:
                         idx_base[h] + (t0 + tcn) * 8],
                    tcn * PB, tcn * PB, EW, single_packet=SINGLE_PACKET,
                    queue_num=qn[0] % NQ)
                qn[0] += 1
                ncall[h] += 1
                for t in range(t0, t0 + tcn):
                    call_of[h][t] = (t0, gt)
                issued[h] += tcn
        for b in range(NB):
            tb0, tb1 = int(TB[0][b]), int(TB[1][b])
            ecol = int(eoff[b])
            tb2 = tb0 + tb1
            agg = pagg.tile([128, D], f32, name=f"agg_{R}_{layer}_{b}",
                            tag="agg")
            # self-loop: own (pre-scaled) rows via identity matmul opens
            # the accumulation group
            ld = dg.tile([128, D], f8, name=f"ld_{R}_{layer}_{b}", tag="ld")
            nc.sync.dma_start(ld[:], own[b * PB:(b + 1) * PB, 0:D])
            nc.tensor.matmul(agg[:], idn8S[:], ld[:, 0:D],
                             start=True, stop=False)
            E = ep.tile([128, tb2, 128], f16,
                        name=f"E_{R}_{layer}_{b}", tag="E")
            iotB = iotS[:, :].unsqueeze(1).broadcast_to((128, tb2, 128))
            dB = dS[:, ecol:ecol + tb2].unsqueeze(2) \
                .broadcast_to((128, tb2, 128))
            nc.vector.tensor_tensor(E[:], iotB, dB,
                                    op=mybir.AluOpType.is_equal)
            mi = 0
            for h in range(2):
                sbase = int(off_h[h][b])
                tbh = tb0 if h == 0 else tb1
                issue_calls(h, sbase + tbh)
                for t in range(tbh):
                    t0, gt = call_of[h][sbase + t]
                    nc.tensor.matmul(
                        agg[:], E[:, (0 if h == 0 else tb0) + t, :],
                        gt[:, sbase + t - t0, 0:D],
                        start=False, stop=(mi == tb2 - 1))
                    mi += 1
                for t in range(sbase, sbase + tbh):
                    call_of[h].pop(t, None)
            # epilogue: dinv[dst] post-scale + transpose, @W, +b, relu, BN
            aggS = sp.tile([128, D], f32, name=f"aggS_{R}_{layer}_{b}",
                           tag="aggS")
            nc.scalar.activation(aggS[:], agg[:],
                                 mybir.ActivationFunctionType.Copy,
                                 scale=dvcS[:, b:b + 1])
            psT = ptr.tile([128, 256], f32, name=f"psT_{R}_{layer}_{b}",
                           tag="psT")
            nc.tensor.transpose(psT[:, 0:128], aggS[:, 0:128], idnS[:])
            nc.tensor.transpose(psT[0:DLO, 128:256], aggS[:, 128:D], idnS[:])
            t1 = sp.tile([128, 128], f16, name=f"t1_{R}_{layer}_{b}", tag="t1")
            nc.scalar.activation(t1[:], psT[:, 0:128],
                                 mybir.ActivationFunctionType.Copy)
            t2 = sp.tile([DLO, 128], f16, name=f"t2_{R}_{layer}_{b}", tag="t2")
            nc.scalar.activation(t2[:], psT[0:DLO, 128:256],
                                 mybir.ActivationFunctionType.Copy)
            zps = ph.tile([128, D], f32, name=f"zps_{R}_{layer}_{b}", tag="zps")
            nc.tensor.matmul(zps[:], t1[:], whiS[layer][:],
                             start=True, stop=False)
            nc.tensor.matmul(zps[:], t2[:], wloS[layer][:],
                             start=False, stop=False)
            nc.tensor.matmul(zps[:], onesS[:], browS[layer][:],
                             start=False, stop=True)
            rl = sp.tile([128, D], f32, name=f"rl_{R}_{layer}_{b}", tag="rl")
            nc.scalar.activation(rl[:], zps[:],
                                 mybir.ActivationFunctionType.Relu)
            hS = sp.tile([128, D], f16, name=f"hS_{R}_{layer}_{b}", tag="hS")
            nc.vector.tensor_add(hS[:], rl[:], betS[layer][:])
            if layer == 0:
                # store row = dinv[dst] * h1, packed to fp8 in a 256B slot
                # (ScalarE does the scale+cast; DVE f8 casts are very slow)
                r1 = sp.tile([128, EW], f8, name=f"r1_{R}_{b}", tag="r1")
                nc.scalar.activation(r1[:, 0:D], hS[:],
                                     mybir.ActivationFunctionType.Copy,
                                     scale=dvcS[:, b:b + 1])
                nc.sync.dma_start(h1sl[b * PB:(b + 1) * PB, 0:D], r1[:, 0:D])
                ends = {int(cumr[i + 1]): i for i in range(len(CBL))}
                e = (b + 1) * PB
                if e in ends:
                    ch = ends[e]
                    r0, re = int(cumr[ch]), int(cumr[ch + 1])
                    nc.gpsimd.collective_compute(
                        "AllGather", mybir.AluOpType.bypass,
                        replica_groups=[list(range(NCORES))],
                        ins=[h1sl[r0:re, :].opt()],
                        outs=[h1f[NCORES * r0:NCORES * re, :].opt()])
            else:
                pps = ppool.tile([128, D], f32, name=f"pps_{R}_{b}", tag="pps")
                nc.tensor.matmul(pps[:], pS[:, b * 128:(b + 1) * 128], hS[:],
                                 start=True, stop=True)
                po = sp.tile([128, D], f32, name=f"po_{R}_{b}", tag="po")
                nc.scalar.activation(po[:], pps[:],
                                     mybir.ActivationFunctionType.Copy)
                nc.sync.dma_start(outp[b * PB:(b + 1) * PB, :], po[:])


# ---------------------------------------------------------------- entry
_NC_CACHE = {}


def kernel(x, edge_index, batch, W1, b1, W2, b2,
           g1, beta1, rm1, rv1, g2, beta2, rm2, rv2):
    nc, in_maps, pp = prepare(x, edge_index, batch, W1, b1, W2, b2,
                              g1, beta1, rm1, rv1, g2, beta2, rm2, rv2)
    res = run_bass_kernel_spmd(nc, in_maps, core_ids=list(range(NCORES)))
    return combine(pp, [res.results[k]["outp"] for k in range(NCORES)])


def prepare(x, edge_index, batch, W1, b1, W2, b2,
            g1, beta1, rm1, rv1, g2, beta2, rm2, rv2):
    """Build (nc, in_maps, pp) without running — used by the benchmark."""
    x = np.asarray(x, dtype=np.float32)
    pp = preprocess(x, np.asarray(edge_index), np.asarray(batch))
    D = pp["D"]
    key = (pp["NB"], D, pp["TOT"], pp["S"], pp["L"],
           tuple(int(v) for v in pp["TB"][0]),
           tuple(int(v) for v in pp["TB"][1]),
           tuple(pp["call_lo"][0]), tuple(pp["call_hi"][0]),
           tuple(pp["call_lo"][1]), tuple(pp["call_hi"][1]))
    if key not in _NC_CACHE:
        _NC_CACHE[key] = build_nc(*key)
    nc = _NC_CACHE[key]

    N = pp["N"]
    TOTp = pp["TOT"]
    xbuf = np.zeros((TOTp, EW), dtype=F8NP)
    xs = (x * pp["dinv"][:N, None]).astype(np.float16).astype(F8NP)
    xbuf[pp["pos_all"][:N], :D] = xs
    xpad = np.zeros((TOTp, EW), dtype=F8NP)
    xpad[:N, :D] = xs                      # node-order copy for xown slices
    iot = np.broadcast_to(np.arange(128, dtype=np.float16), (128, 128)).copy()
    idn = np.eye(128, dtype=np.float32)
    idn8 = np.eye(128, dtype=np.float32).astype(F8NP)
    onesr = np.ones((1, 128), dtype=np.float16)
    g1p, b1p = fold_bn(g1, beta1, rm1, rv1)
    g2p, b2p = fold_bn(g2, beta2, rm2, rv2)
    assert (g1p > 0).all() and (g2p > 0).all(), \
        "gamma fold into W needs positive BN scale (relu(z*g) == relu(z)*g)"
    whi = np.stack([W1[:128] * g1p, W2[:128] * g2p]).astype(np.float16)
    wlo = np.stack([W1[128:] * g1p, W2[128:] * g2p]).astype(np.float16)
    brow = np.stack([(b1 * g1p)[None, :],
                     (b2 * g2p)[None, :]]).astype(np.float16)
    bet = np.stack([np.broadcast_to(b1p, (128, D)),
                    np.broadcast_to(b2p, (128, D))]).astype(np.float32)
    in_maps = []
    for k in range(NCORES):
        in_maps.append({
            "xbuf": xbuf, "xown": xpad[k * pp["S"]:(k + 1) * pp["S"]],
            "idxt": pp["idx_arr"][k], "dcol": pp["d_arr"][k],
            "dvc": pp["dinvc_arr"][k],
            "pcol": pp["p_arr"][k],
            "iot": iot, "idn": idn, "idn8": idn8, "onesr": onesr,
            "whi": whi, "wlo": wlo, "brow": brow, "bet": bet,
        })
    return nc, in_maps, pp


def combine(pp, outs):
    sums = np.zeros((pp["G"] + PB, pp["D"]), dtype=np.float32)
    for k in range(NCORES):
        o = outs[k]
        for b in range(pp["NB"]):
            bb = pp["base"][k * pp["NB"] + b]
            sums[bb:bb + PB] += o[b * PB:(b + 1) * PB]
    return (sums[:pp["G"]]
            / np.maximum(pp["cnts"], 1.0)[:, None]).astype(np.float32)
